# revision 13
# baseline (speedup 1.0000x reference)
"""Multi-head attention (B=2, S=2048, D=1024, H=16, dk=64) on 8 trn2 cores.

Sharding: core c handles batch b=c//4 and 4 heads g=c%4 (heads 4g..4g+3).
Each core computes its heads' Q/K/V projections, attention, and a partial
output projection; the host sums the 4 partials per batch.

Per-core kernel layout (everything contracted on partitions for the PE):
  - host pre-transposes x -> xT [D, S] so projections contract over D.
  - qhT/khT [256 head-dims, S] (bf16), vh [S, 256] natural (bf16).
  - scores tile [128 q, 2048 k] in PSUM (fp32), one (head, qtile) at a time.
  - softmax: no max-subtraction (scores ~ N(0,1), |s| < ~6 is safe in fp32);
    exp on ScalarE with fused accum_out giving the row sums; reciprocal +
    per-partition scale on VectorE.
  - normalized attn is DMA-xbar-transposed (bf16) to [k, q] layout so the
    PV matmul contracts k on partitions with N=512 moving operand.
  - bk/bv/bo never touch the device: bk only shifts scores by a per-row
    constant (softmax-invariant), bv/bo fold into a host-side correction
    bv @ wo.T + bo added once per batch. bq is added on-device (scaled by
    1/sqrt(dk), which is folded into wq/bq on the host).
"""

import sys

for _p in ("/opt/trn_rl_repo",):
    if _p not in sys.path:
        sys.path.insert(0, _p)

from contextlib import ExitStack

import ml_dtypes
import numpy as np

import concourse.bass as bass
import concourse.bacc as bacc_mod
import concourse.mybir as mybir
import concourse.tile as tile
from concourse.bass_utils import run_bass_kernel_spmd

BF16 = mybir.dt.bfloat16
F32 = mybir.dt.float32
AF = mybir.ActivationFunctionType

B, S, D = 2, 2048, 1024
NCORES = 8
HLOC = 4          # heads per core
DK = 64
HD = HLOC * DK    # local head dims = 256
KT_D = D // 128   # 8 k-tiles over the model dim
NST = S // 128    # 16 tiles over sequence
NQG = 4           # q groups of 512
NKB = 4           # k blocks of 512


def build_nc() -> bass.Bass:
    nc = bacc_mod.Bacc()

    xqT = nc.dram_tensor("xqT", [D, S], BF16, kind="ExternalInput")
    xkT = nc.dram_tensor("xkT", [D, S], BF16, kind="ExternalInput")
    xvT = nc.dram_tensor("xvT", [D, S], BF16, kind="ExternalInput")
    wqT = nc.dram_tensor("wqT", [D, HD], BF16, kind="ExternalInput")
    wkT = nc.dram_tensor("wkT", [D, HD], BF16, kind="ExternalInput")
    wvT = nc.dram_tensor("wvT", [D, HD], BF16, kind="ExternalInput")
    woT = nc.dram_tensor("woT", [HD, D], BF16, kind="ExternalInput")
    bqd = nc.dram_tensor("bq", [HD], BF16, kind="ExternalInput")
    out = nc.dram_tensor("out_partial", [S, D], F32, kind="ExternalOutput")

    with tile.TileContext(nc) as tc, ExitStack() as ctx:
        const = ctx.enter_context(tc.tile_pool(name="const", bufs=1))
        persist = ctx.enter_context(tc.tile_pool(name="persist", bufs=1))

        # Weights / bias resident in SBUF.
        wq_s = const.tile([128, KT_D, HD], BF16, tag="wq")
        wk_s = const.tile([128, KT_D, HD], BF16, tag="wk")
        wv_s = const.tile([128, KT_D, HD], BF16, tag="wv")
        wo_s = const.tile([128, 2, D], BF16, tag="wo")
        bq_s = const.tile([1, HD], BF16, tag="bq")
        ones_row = const.tile([1, S], BF16, tag="ones")
        nc.vector.memset(ones_row, 1.0)
        nc.sync.dma_start(wq_s, wqT.rearrange("(t p) n -> p t n", p=128))
        nc.sync.dma_start(wk_s, wkT.rearrange("(t p) n -> p t n", p=128))
        nc.sync.dma_start(wv_s, wvT.rearrange("(t p) n -> p t n", p=128))
        nc.sync.dma_start(wo_s, woT.rearrange("(t p) n -> p t n", p=128))
        nc.sync.dma_start(bq_s, bqd.rearrange("(o n) -> o n", o=1))

        # Projection outputs resident in SBUF.
        qhT = [persist.tile([128, S], BF16, tag=f"qhT{m}", name=f"qhT{m}") for m in range(2)]
        khT = [persist.tile([128, S], BF16, tag=f"khT{m}", name=f"khT{m}") for m in range(2)]
        vh_s = persist.tile([128, NST, HD], BF16, tag="vh")
        ctxT = [persist.tile([128, S], BF16, tag=f"ctxT{m}", name=f"ctxT{m}") for m in range(2)]

        # ---- Phase 1: projections -------------------------------------
        with tc.tile_pool(name="xload", bufs=2) as xpool:
          with tc.tile_pool(name="qk_psum", bufs=2, space="PSUM") as qkp:
            xq_t = xpool.tile([128, KT_D, S], BF16, tag="x")
            nc.sync.dma_start(xq_t, xqT.rearrange("(t p) n -> p t n", p=128))
            for m in range(2):
                ps = qkp.tile([128, S], F32, tag="proj")
                for kt in range(KT_D):
                    for nb in range(NKB):
                        nc.tensor.matmul(
                            ps[:, nb * 512 : (nb + 1) * 512],
                            lhsT=wq_s[:, kt, m * 128 : (m + 1) * 128],
                            rhs=xq_t[:, kt, nb * 512 : (nb + 1) * 512],
                            start=(kt == 0),
                            stop=False,
                        )
                # bias as rank-1 matmul: qhT[p, n] += bq[p] * 1
                for nb in range(NKB):
                    nc.tensor.matmul(
                        ps[:, nb * 512 : (nb + 1) * 512],
                        lhsT=bq_s[0:1, m * 128 : (m + 1) * 128],
                        rhs=ones_row[0:1, nb * 512 : (nb + 1) * 512],
                        start=False,
                        stop=True,
                    )
                nc.vector.tensor_copy(qhT[m], ps)

            xk_t = xpool.tile([128, KT_D, S], BF16, tag="x")
            nc.sync.dma_start(xk_t, xkT.rearrange("(t p) n -> p t n", p=128))
            for m in range(2):
                ps = qkp.tile([128, S], F32, tag="proj")
                for kt in range(KT_D):
                    for nb in range(NKB):
                        nc.tensor.matmul(
                            ps[:, nb * 512 : (nb + 1) * 512],
                            lhsT=wk_s[:, kt, m * 128 : (m + 1) * 128],
                            rhs=xk_t[:, kt, nb * 512 : (nb + 1) * 512],
                            start=(kt == 0),
                            stop=(kt == KT_D - 1),
                        )
                nc.vector.tensor_copy(khT[m], ps)

          xv_t = xpool.tile([128, KT_D, S], BF16, tag="x")
          nc.sync.dma_start(xv_t, xvT.rearrange("(t p) n -> p t n", p=128))
          with tc.tile_pool(name="v_psum", bufs=4, space="PSUM") as vps:
                for st in range(NST):
                    ps = vps.tile([128, HD], F32, tag="vproj")
                    for kt in range(KT_D):
                        nc.tensor.matmul(
                            ps,
                            lhsT=xv_t[:, kt, st * 128 : (st + 1) * 128],
                            rhs=wv_s[:, kt, :],
                            start=(kt == 0),
                            stop=(kt == KT_D - 1),
                        )
                    nc.vector.tensor_copy(vh_s[:, st, :], ps)

        # ---- Phase 2: attention ---------------------------------------
        with (
            tc.tile_pool(name="sc_psum", bufs=1, space="PSUM") as scp,
            tc.tile_pool(name="ctx_psum", bufs=2, space="PSUM") as ctxp,
            tc.tile_pool(name="exp_pool", bufs=3) as epool,
            tc.tile_pool(name="attnT_pool", bufs=2) as atp,
            tc.tile_pool(name="stat_pool", bufs=4) as stp,
        ):
            for hp in range(2):          # head pairs -> ctxT tile hp
                for qg in range(NQG):    # 512-wide q groups
                    ctx_ps = ctxp.tile([128, 512], F32, tag="ctx")
                    for e in range(2):   # head within pair
                        po = e * 64      # partition offset of this head
                        attnT_t = atp.tile([128, NQG, NST, 128], BF16, tag="attnT")
                        for qt in range(4):
                            qtile = qg * 4 + qt
                            sc = scp.tile([128, S], F32, tag="scores")
                            for kb in range(NKB):
                                nc.tensor.matmul(
                                    sc[:, kb * 512 : (kb + 1) * 512],
                                    lhsT=qhT[hp][po : po + 64, qtile * 128 : (qtile + 1) * 128],
                                    rhs=khT[hp][po : po + 64, kb * 512 : (kb + 1) * 512],
                                    start=True,
                                    stop=True,
                                )
                            exp_t = epool.tile([128, S], BF16, tag="exp")
                            ssum = stp.tile([128, 1], F32, tag="ssum")
                            nc.scalar.activation(exp_t, sc, AF.Exp, accum_out=ssum)
                            rsum = stp.tile([128, 1], F32, tag="rsum")
                            nc.vector.reciprocal(rsum, ssum)
                            # normalize in place (single sync wait for ptr op)
                            nc.vector.tensor_scalar_mul(exp_t, exp_t, rsum)
                            nc.sync.dma_start_transpose(attnT_t[:, qt], exp_t)
                        # PV: ctxT_h[dk, 512 q] += vh_kt.T @ attnT_kt
                        h = hp * 2 + e
                        for kt in range(NST):
                            nc.tensor.matmul(
                                ctx_ps[po : po + 64, :],
                                lhsT=vh_s[:, kt, h * 64 : (h + 1) * 64],
                                rhs=attnT_t[:, :, kt, :],
                                start=(kt == 0),
                                stop=(kt == NST - 1),
                                tile_position=(0, po),
                            )
                    nc.vector.tensor_copy(
                        ctxT[hp][:, qg * 512 : (qg + 1) * 512], ctx_ps
                    )

        # ---- Phase 3: output projection -------------------------------
        with (
            tc.tile_pool(name="out_psum", bufs=2, space="PSUM") as outp,
            tc.tile_pool(name="out_sbuf", bufs=2) as outs,
        ):
            for st in range(NST):
                ops = outp.tile([128, D], F32, tag="out")
                for kt in range(2):
                    for nb in range(2):
                        nc.tensor.matmul(
                            ops[:, nb * 512 : (nb + 1) * 512],
                            lhsT=ctxT[kt][:, st * 128 : (st + 1) * 128],
                            rhs=wo_s[:, kt, nb * 512 : (nb + 1) * 512],
                            start=(kt == 0),
                            stop=(kt == 1),
                        )
                ob = outs.tile([128, D], F32, tag="ob")
                nc.vector.tensor_copy(ob, ops)
                nc.sync.dma_start(out[st * 128 : (st + 1) * 128, :], ob)

    nc.compile()
    return nc


_CACHE: dict = {}


def _get_nc() -> bass.Bass:
    if "nc" not in _CACHE:
        _CACHE["nc"] = build_nc()
    return _CACHE["nc"]


def _bf16(x: np.ndarray) -> np.ndarray:
    return np.ascontiguousarray(x).astype(ml_dtypes.bfloat16)


def make_in_maps(q, k, v, wq, bq, wk, bk, wv, bv, wo, bo):
    scale = np.float32(1.0 / np.sqrt(DK))
    in_maps = []
    for c in range(NCORES):
        b, g = divmod(c, 4)
        hh = g * HD
        in_maps.append(
            {
                "xqT": _bf16(np.asarray(q[b], np.float32).T),
                "xkT": _bf16(np.asarray(k[b], np.float32).T),
                "xvT": _bf16(np.asarray(v[b], np.float32).T),
                "wqT": _bf16(np.asarray(wq[hh : hh + HD], np.float32).T * scale),
                "wkT": _bf16(np.asarray(wk[hh : hh + HD], np.float32).T),
                "wvT": _bf16(np.asarray(wv[hh : hh + HD], np.float32).T),
                "woT": _bf16(np.asarray(wo[:, hh : hh + HD], np.float32).T),
                "bq": _bf16(np.asarray(bq[hh : hh + HD], np.float32) * scale),
            }
        )
    return in_maps


def assemble(results, bv, bo, wo) -> np.ndarray:
    out = np.zeros((B, S, D), np.float32)
    for c in range(NCORES):
        out[c // 4] += np.asarray(results[c]["out_partial"], np.float32)
    corr = np.asarray(bv, np.float32) @ np.asarray(wo, np.float32).T + np.asarray(
        bo, np.float32
    )
    out += corr[None, None, :]
    return out


def kernel(q, k, v, wq, bq, wk, bk, wv, bv, wo, bo) -> np.ndarray:
    nc = _get_nc()
    in_maps = make_in_maps(q, k, v, wq, bq, wk, bk, wv, bv, wo, bo)
    res = run_bass_kernel_spmd(nc, in_maps, list(range(NCORES))).results
    return assemble(res, bv, bo, wo)


# revision 15
# speedup vs baseline: 1.0982x; 1.0982x over previous
"""Multi-head attention (B=2, S=2048, D=1024, H=16, dk=64) on 8 trn2 cores.

Sharding: core c handles batch b=c//4 and 4 heads g=c%4 (heads 4g..4g+3).
Each core computes its heads' Q/K/V projections, attention, and a partial
output projection; the host sums the 4 partials per batch.

Per-core kernel layout (everything contracted on partitions for the PE):
  - host pre-transposes x -> xT [D, S] so projections contract over D.
  - qhT/khT [256 head-dims, S] (bf16), vh [S, 256] natural (bf16).
  - scores tile [128 q, 2048 k] in PSUM (fp32), one (head, qtile) at a time.
  - softmax: no max-subtraction (scores ~ N(0,1), |s| < ~6 is safe in fp32);
    exp on ScalarE with fused accum_out giving the row sums; reciprocal +
    per-partition scale on VectorE.
  - normalized attn is DMA-xbar-transposed (bf16) to [k, q] layout so the
    PV matmul contracts k on partitions with N=512 moving operand.
  - bk/bv/bo never touch the device: bk only shifts scores by a per-row
    constant (softmax-invariant), bv/bo fold into a host-side correction
    bv @ wo.T + bo added once per batch. bq is added on-device (scaled by
    1/sqrt(dk), which is folded into wq/bq on the host).
"""

import sys

for _p in ("/opt/trn_rl_repo",):
    if _p not in sys.path:
        sys.path.insert(0, _p)

from contextlib import ExitStack

import ml_dtypes
import numpy as np

import concourse.bass as bass
import concourse.bacc as bacc_mod
import concourse.mybir as mybir
import concourse.tile as tile
from concourse.bass_utils import run_bass_kernel_spmd

BF16 = mybir.dt.bfloat16
F32 = mybir.dt.float32
AF = mybir.ActivationFunctionType

B, S, D = 2, 2048, 1024
NCORES = 8
HLOC = 4          # heads per core
DK = 64
HD = HLOC * DK    # local head dims = 256
KT_D = D // 128   # 8 k-tiles over the model dim
NST = S // 128    # 16 tiles over sequence
NQG = 4           # q groups of 512
NKB = 4           # k blocks of 512


def build_nc() -> bass.Bass:
    nc = bacc_mod.Bacc()

    xqT = nc.dram_tensor("xqT", [D, S], BF16, kind="ExternalInput")
    xkT = nc.dram_tensor("xkT", [D, S], BF16, kind="ExternalInput")
    xvT = nc.dram_tensor("xvT", [D, S], BF16, kind="ExternalInput")
    wqT = nc.dram_tensor("wqT", [D, HD], BF16, kind="ExternalInput")
    wkT = nc.dram_tensor("wkT", [D, HD], BF16, kind="ExternalInput")
    wvT = nc.dram_tensor("wvT", [D, HD], BF16, kind="ExternalInput")
    woT = nc.dram_tensor("woT", [HD, D], BF16, kind="ExternalInput")
    bqd = nc.dram_tensor("bq", [HD], BF16, kind="ExternalInput")
    out = nc.dram_tensor("out_partial", [S, D], F32, kind="ExternalOutput")

    with tile.TileContext(nc) as tc, ExitStack() as ctx:
        const = ctx.enter_context(tc.tile_pool(name="const", bufs=1))
        persist = ctx.enter_context(tc.tile_pool(name="persist", bufs=1))

        # Weights / bias resident in SBUF.
        wq_s = const.tile([128, KT_D, HD], BF16, tag="wq")
        wk_s = const.tile([128, KT_D, HD], BF16, tag="wk")
        wv_s = const.tile([128, KT_D, HD], BF16, tag="wv")
        wo_s = const.tile([128, 2, D], BF16, tag="wo")
        bq_s = const.tile([1, HD], BF16, tag="bq")
        ones_row = const.tile([1, S], BF16, tag="ones")
        nc.vector.memset(ones_row, 1.0)
        nc.sync.dma_start(wq_s, wqT.rearrange("(t p) n -> p t n", p=128))
        nc.sync.dma_start(wk_s, wkT.rearrange("(t p) n -> p t n", p=128))
        nc.sync.dma_start(wv_s, wvT.rearrange("(t p) n -> p t n", p=128))
        nc.sync.dma_start(wo_s, woT.rearrange("(t p) n -> p t n", p=128))
        nc.sync.dma_start(bq_s, bqd.rearrange("(o n) -> o n", o=1))

        # Projection outputs resident in SBUF.
        qhT = [persist.tile([128, S], BF16, tag=f"qhT{m}", name=f"qhT{m}") for m in range(2)]
        khT = [persist.tile([128, S], BF16, tag=f"khT{m}", name=f"khT{m}") for m in range(2)]
        vh_s = persist.tile([128, NST, HD], BF16, tag="vh")
        ctxT = [persist.tile([128, S], BF16, tag=f"ctxT{m}", name=f"ctxT{m}") for m in range(2)]

        # ---- Phase 1: projections -------------------------------------
        with tc.tile_pool(name="xload", bufs=2) as xpool:
          with tc.tile_pool(name="qk_psum", bufs=2, space="PSUM") as qkp:
            xq_t = xpool.tile([128, KT_D, S], BF16, tag="x")
            for _kt in range(KT_D):
                nc.sync.dma_start(xq_t[:, _kt], xqT[_kt * 128 : (_kt + 1) * 128, :])
            for m in range(2):
                ps = qkp.tile([128, S], F32, tag="proj")
                for kt in range(KT_D):
                    for nb in range(NKB):
                        nc.tensor.matmul(
                            ps[:, nb * 512 : (nb + 1) * 512],
                            lhsT=wq_s[:, kt, m * 128 : (m + 1) * 128],
                            rhs=xq_t[:, kt, nb * 512 : (nb + 1) * 512],
                            start=(kt == 0),
                            stop=False,
                        )
                # bias as rank-1 matmul: qhT[p, n] += bq[p] * 1
                for nb in range(NKB):
                    nc.tensor.matmul(
                        ps[:, nb * 512 : (nb + 1) * 512],
                        lhsT=bq_s[0:1, m * 128 : (m + 1) * 128],
                        rhs=ones_row[0:1, nb * 512 : (nb + 1) * 512],
                        start=False,
                        stop=True,
                    )
                nc.vector.tensor_copy(qhT[m], ps)

            xk_t = xpool.tile([128, KT_D, S], BF16, tag="x")
            for _kt in range(KT_D):
                nc.sync.dma_start(xk_t[:, _kt], xkT[_kt * 128 : (_kt + 1) * 128, :])
            for m in range(2):
                ps = qkp.tile([128, S], F32, tag="proj")
                for kt in range(KT_D):
                    for nb in range(NKB):
                        nc.tensor.matmul(
                            ps[:, nb * 512 : (nb + 1) * 512],
                            lhsT=wk_s[:, kt, m * 128 : (m + 1) * 128],
                            rhs=xk_t[:, kt, nb * 512 : (nb + 1) * 512],
                            start=(kt == 0),
                            stop=(kt == KT_D - 1),
                        )
                nc.vector.tensor_copy(khT[m], ps)

          xv_t = xpool.tile([128, KT_D, S], BF16, tag="x")
          for _kt in range(KT_D):
              nc.sync.dma_start(xv_t[:, _kt], xvT[_kt * 128 : (_kt + 1) * 128, :])
          with tc.tile_pool(name="v_psum", bufs=4, space="PSUM") as vps:
                for st in range(NST):
                    ps = vps.tile([128, HD], F32, tag="vproj")
                    for kt in range(KT_D):
                        nc.tensor.matmul(
                            ps,
                            lhsT=xv_t[:, kt, st * 128 : (st + 1) * 128],
                            rhs=wv_s[:, kt, :],
                            start=(kt == 0),
                            stop=(kt == KT_D - 1),
                        )
                    nc.vector.tensor_copy(vh_s[:, st, :], ps)

        # ---- Phase 2+3: attention fused with output projection ------
        # qg outer so each q-group's output projection overlaps the next
        # q-group's attention. Scores are computed in two 1024-wide halves
        # (2 PSUM banks each, separate tags) so the PE can refill one half
        # while ScalarE exponentiates the other — keeps the PE dense enough
        # that HAM stays at K=8/8.
        with (
            tc.tile_pool(name="scA_psum", bufs=1, space="PSUM") as scpa,
            tc.tile_pool(name="scB_psum", bufs=1, space="PSUM") as scpb,
            tc.tile_pool(name="ctx_psum", bufs=2, space="PSUM") as ctxp,
            tc.tile_pool(name="out_psum", bufs=2, space="PSUM") as outp,
            tc.tile_pool(name="exp_pool", bufs=3) as epool,
            tc.tile_pool(name="attnT_pool", bufs=2) as atp,
            tc.tile_pool(name="stat_pool", bufs=8) as stp,
            tc.tile_pool(name="out_sbuf", bufs=3) as outs,
        ):
            for qg in range(NQG):        # 512-wide q groups
                for hp in range(2):      # head pairs -> ctxT tile hp
                    ctx_ps = ctxp.tile([128, 512], F32, tag="ctx")
                    for e in range(2):   # head within pair
                        po = e * 64      # partition offset of this head
                        attnT_t = atp.tile([128, 4, NST, 128], BF16, tag="attnT")
                        for qt in range(4):
                            qtile = qg * 4 + qt
                            exp_t = epool.tile([128, S], BF16, tag="exp")
                            acc = [None, None]
                            for half, scpool in ((0, scpa), (1, scpb)):
                                sc = scpool.tile([128, 1024], F32, tag=f"sc{half}")
                                for kb in range(2):
                                    nc.tensor.matmul(
                                        sc[:, kb * 512 : (kb + 1) * 512],
                                        lhsT=qhT[hp][po : po + 64, qtile * 128 : (qtile + 1) * 128],
                                        rhs=khT[hp][po : po + 64, (half * 2 + kb) * 512 : (half * 2 + kb + 1) * 512],
                                        start=True,
                                        stop=True,
                                    )
                                a = stp.tile([128, 1], F32, tag=f"acc{half}", name=f"acc{half}")
                                nc.scalar.activation(
                                    exp_t[:, half * 1024 : (half + 1) * 1024],
                                    sc, AF.Exp, accum_out=a,
                                )
                                acc[half] = a
                            rsum = stp.tile([128, 1], F32, tag="rsum")
                            nc.vector.tensor_add(rsum, acc[0], acc[1])
                            nc.vector.reciprocal(rsum, rsum)
                            for half in range(2):
                                nc.vector.tensor_scalar_mul(
                                    exp_t[:, half * 1024 : (half + 1) * 1024],
                                    exp_t[:, half * 1024 : (half + 1) * 1024],
                                    rsum,
                                )
                                nc.sync.dma_start_transpose(
                                    attnT_t[:, qt, half * 8 : (half + 1) * 8, :],
                                    exp_t[:, half * 1024 : (half + 1) * 1024],
                                )
                        # PV: ctxT_h[dk, 512 q] += vh_kt.T @ attnT_kt
                        h = hp * 2 + e
                        for kt in range(NST):
                            nc.tensor.matmul(
                                ctx_ps[po : po + 64, :],
                                lhsT=vh_s[:, kt, h * 64 : (h + 1) * 64],
                                rhs=attnT_t[:, :, kt, :],
                                start=(kt == 0),
                                stop=(kt == NST - 1),
                                tile_position=(0, po),
                            )
                    nc.vector.tensor_copy(
                        ctxT[hp][:, qg * 512 : (qg + 1) * 512], ctx_ps
                    )
                # output projection for this q-group's 4 S-tiles
                for st in range(qg * 4, qg * 4 + 4):
                    ob = outs.tile([128, D], F32, tag="ob")
                    for nb in range(2):
                        ops = outp.tile([128, 512], F32, tag="out")
                        for kt in range(2):
                            nc.tensor.matmul(
                                ops,
                                lhsT=ctxT[kt][:, st * 128 : (st + 1) * 128],
                                rhs=wo_s[:, kt, nb * 512 : (nb + 1) * 512],
                                start=(kt == 0),
                                stop=(kt == 1),
                            )
                        nc.vector.tensor_copy(ob[:, nb * 512 : (nb + 1) * 512], ops)
                    nc.sync.dma_start(out[st * 128 : (st + 1) * 128, :], ob)

    nc.compile()
    return nc


_CACHE: dict = {}


def _get_nc() -> bass.Bass:
    if "nc" not in _CACHE:
        _CACHE["nc"] = build_nc()
    return _CACHE["nc"]


def _bf16(x: np.ndarray) -> np.ndarray:
    return np.ascontiguousarray(x).astype(ml_dtypes.bfloat16)


def make_in_maps(q, k, v, wq, bq, wk, bk, wv, bv, wo, bo):
    scale = np.float32(1.0 / np.sqrt(DK))
    in_maps = []
    for c in range(NCORES):
        b, g = divmod(c, 4)
        hh = g * HD
        in_maps.append(
            {
                "xqT": _bf16(np.asarray(q[b], np.float32).T),
                "xkT": _bf16(np.asarray(k[b], np.float32).T),
                "xvT": _bf16(np.asarray(v[b], np.float32).T),
                "wqT": _bf16(np.asarray(wq[hh : hh + HD], np.float32).T * scale),
                "wkT": _bf16(np.asarray(wk[hh : hh + HD], np.float32).T),
                "wvT": _bf16(np.asarray(wv[hh : hh + HD], np.float32).T),
                "woT": _bf16(np.asarray(wo[:, hh : hh + HD], np.float32).T),
                "bq": _bf16(np.asarray(bq[hh : hh + HD], np.float32) * scale),
            }
        )
    return in_maps


def assemble(results, bv, bo, wo) -> np.ndarray:
    out = np.zeros((B, S, D), np.float32)
    for c in range(NCORES):
        out[c // 4] += np.asarray(results[c]["out_partial"], np.float32)
    corr = np.asarray(bv, np.float32) @ np.asarray(wo, np.float32).T + np.asarray(
        bo, np.float32
    )
    out += corr[None, None, :]
    return out


def kernel(q, k, v, wq, bq, wk, bk, wv, bv, wo, bo) -> np.ndarray:
    nc = _get_nc()
    in_maps = make_in_maps(q, k, v, wq, bq, wk, bk, wv, bv, wo, bo)
    res = run_bass_kernel_spmd(nc, in_maps, list(range(NCORES))).results
    return assemble(res, bv, bo, wo)
